# revision 21
# baseline (speedup 1.0000x reference)
"""Trainium2 Bass kernel for nn_CDC_62646392980082 (GRU-CPC loss_fn), v2.

Contract: kernel(**inputs) takes the FULL unsharded inputs (numpy) and
returns the FULL output (loss, acc) exactly like the jax reference.

Strategy (8 NeuronCores, data-parallel over batch B=256 -> 32/core):
  - GRU gates fused in PSUM: gi (x@W_ih) and gh (h@W_hh) accumulate into
    the same PSUM bank per step; sigmoid reads PSUM directly on the
    scalar engine (no gi copies / adds on the DVE).
  - H' = h+1 reparameterization: h' = (n+1)(1-z) + z*h' with n+1 =
    2*sigmoid(2x); rank-1 corrections folded into biases host-side.
    Avoids tanh table swaps and one DVE op per gate step.
  - preds split into two passes by r: pass0 = {k2:r0-3, k3:r0-2,
    k4:r0-1} (63 rows), pass1 = {k0:r0-5, k1:r0-4} (77 rows), so
    pass0's dots can overlap the preds tail.
  - clip alternates engines per p-chunk: even chunks clip directly from
    PSUM on the DVE; odd chunks evacuate via scalar-engine Identity and
    clip fp16->fp16 on the DVE fast path.
  - negatives folded host-side into multiplicity counts; the cnt==0
    mask (-60000) is folded into the corr tensor; softmax shift uses a
    per-partition-row max so exp's bias port applies it for free.
"""

import sys

if "/opt/trn_rl_repo" not in sys.path:
    sys.path.insert(0, "/opt/trn_rl_repo")

import numpy as np

B, K, R, C, P, H, S = 256, 5, 6, 7, 1280, 256, 64
NCORE = 8
BS = B // NCORE            # 32 images per core
BC = BS * C                # 224 (b, c) columns
PC_N = P // 128            # 10 p-chunks
HC_N = H // 128            # 2 h-chunks
IJ = 49                    # 7x7 cells

# pass structure: pairs (k, r) grouped so pass0 finishes by GRU step 3
PASS_PAIRS = [
    [(2, 0), (2, 1), (2, 2), (2, 3), (3, 0), (3, 1), (3, 2), (4, 0), (4, 1)],
    [(0, 0), (0, 1), (0, 2), (0, 3), (0, 4), (0, 5),
     (1, 0), (1, 1), (1, 2), (1, 3), (1, 4)],
]
PR = [len(PASS_PAIRS[0]) * C, len(PASS_PAIRS[1]) * C]   # 63, 77 rows
ROW_OFF = {}
for _pi, _lst in enumerate(PASS_PAIRS):
    for _qi, _kr in enumerate(_lst):
        ROW_OFF[_kr] = (_pi, _qi * C)

# preds chunks: (pass, k, [r...]) with adjacent r, emitted after step max(r)
CHUNKS = [
    (0, 4, [0, 1]), (0, 3, [0, 1]),            # ready after step 1
    (0, 3, [2]), (1, 1, [0, 1]),               # after step 2
    (0, 2, [0, 1]), (0, 2, [2, 3]),
    (1, 0, [0, 1]), (1, 0, [2, 3]), (1, 1, [2, 3]),   # after step 3
    (1, 1, [4]),                               # after step 4
    (1, 0, [4, 5]),                            # after step 5
]
N_PREDS = 20 * B * C       # 35840 global predictions
MASK = np.float32(-60000.0)

_CACHE = {}


def _build_program():
    import concourse.bacc as bacc
    import concourse.mybir as mybir
    from concourse.tile import TileContext

    f32 = mybir.dt.float32
    f16 = mybir.dt.float16
    Alu = mybir.AluOpType
    Act = mybir.ActivationFunctionType
    AxX = mybir.AxisListType.X

    nc = bacc.Bacc()
    dp = nc.declare_dram_parameter
    encT = dp("encT", [128, R * PC_N * BC], f16, isOutput=False)   # r-major
    encB = dp("encB", [128, PC_N * BS * IJ], f16, isOutput=False)
    wih = dp("wih", [128, PC_N * 768], f16, isOutput=False)
    whh = dp("whh", [128, HC_N * 768], f16, isOutput=False)
    wk = dp("wk", [K, 128, HC_N * P], f16, isOutput=False)
    brz = dp("brz", [128, 4], f32, isOutput=False)     # r/z bias (H'-folded)
    nbrz = dp("nbrz", [128, 2], f32, isOutput=False)   # negated z bias
    bhn = dp("bhn", [128, 2], f32, isOutput=False)     # h-side n bias
    bin_ = dp("bin", [128, 2], f32, isOutput=False)    # x-side n bias
    wklo = dp("wklo", [128, K * PC_N], f32, isOutput=False)
    wkhi = dp("wkhi", [128, K * PC_N], f32, isOutput=False)
    corr0 = dp("corr0", [PR[0], BS * IJ], f16, isOutput=False)  # corr - mask
    corr1 = dp("corr1", [PR[1], BS * IJ], f16, isOutput=False)
    cnt0 = dp("cnt0", [PR[0], BS * IJ], f16, isOutput=False)
    cnt1 = dp("cnt1", [PR[1], BS * IJ], f16, isOutput=False)
    posm0 = dp("posm0", [PR[0], IJ], f16, isOutput=False)
    posm1 = dp("posm1", [PR[1], IJ], f16, isOutput=False)
    # per (pass, half): [se 16][pos 16][mxg 16][mxp 1] in 49-col blocks
    out0 = dp("out0", [PR[0], 2 * IJ], f32, isOutput=True)
    out1 = dp("out1", [PR[1], 2 * IJ], f32, isOutput=True)
    corr_d = [corr0, corr1]
    cnt_d = [cnt0, cnt1]
    posm_d = [posm0, posm1]
    out_d = [out0, out1]

    with TileContext(nc, pool_alloc_mode="queue") as tc:
        with tc.tile_pool(name="pers", bufs=1) as pers:
            # ---- persistent small loads (sync queue) ----
            brz_t = pers.tile([128, 4], f32)
            nc.sync.dma_start(out=brz_t, in_=brz[:, :])
            nbrz_t = pers.tile([128, 2], f32)
            nc.sync.dma_start(out=nbrz_t, in_=nbrz[:, :])
            bhn_t = pers.tile([128, 2], f32)
            nc.sync.dma_start(out=bhn_t, in_=bhn[:, :])
            bin_t = pers.tile([128, 2], f32)
            nc.sync.dma_start(out=bin_t, in_=bin_[:, :])
            wklo_t = pers.tile([128, K * PC_N], f32)
            nc.sync.dma_start(out=wklo_t, in_=wklo[:, :])
            wkhi_t = pers.tile([128, K * PC_N], f32)
            nc.sync.dma_start(out=wkhi_t, in_=wkhi[:, :])
            posm_t = [pers.tile([PR[pi], IJ], f16, name=f"posm{pi}") for pi in range(2)]
            for pi in range(2):
                nc.sync.dma_start(out=posm_t[pi], in_=posm_d[pi][:, :])

            # dots-phase encodings: loaded early on the gpsimd/scalar queues
            encB_b = pers.tile([128, PC_N * BS * IJ], f16, name="encB_b")
            hw_ = PC_N * BS * IJ // 2
            nc.gpsimd.dma_start(out=encB_b[:, hw_:], in_=encB[:, hw_:])

            # GRU context (H' = h+1), per h-chunk: [128, r*224]
            ctx = [pers.tile([128, R * BC], f16, name=f"ctx{t}") for t in range(2)]
            h0 = pers.tile([128, BC], f16)
            nc.vector.memset(h0, 1.0)

            predsT = [
                [
                    pers.tile([128, BS * PR[pi]], f16, name=f"pt{pi}_{m}")
                    for m in range(PC_N)
                ]
                for pi in range(2)
            ]

            # scratch pools
            scr = tc.alloc_tile_pool(name="scr", bufs=1)
            wkp = tc.alloc_tile_pool(name="wkp", bufs=1)
            psPP = tc.alloc_tile_pool(name="psPP", bufs=3, space="PSUM")

            wk_t = {}

            def load_wk(k):
                t = wkp.tile([128, HC_N * P], f16, tag="wk", bufs=3, name=f"wk{k}")
                nc.sync.dma_start(out=t, in_=wk[k, :, :])
                wk_t[k] = t

            # ---------- preds emission ----------
            def emit_preds_chunk(pi, k, rs):
                nq = len(rs)
                n = nq * BC
                for m in range(PC_N):
                    ps = psPP.tile([128, 512], f32, tag="pp", name=f"pp{pi}_{k}_{rs[0]}_{m}")
                    for hc in range(HC_N):
                        nc.tensor.matmul(
                            ps[:, :n],
                            wk_t[k][:, hc * P + m * 128 : hc * P + (m + 1) * 128],
                            ctx[hc][:, rs[0] * BC : (rs[0] + nq) * BC],
                            start=(hc == 0),
                            stop=(hc == HC_N - 1),
                        )
                    off = ROW_OFF[(k, rs[0])][1]
                    dst = (
                        predsT[pi][m]
                        .rearrange("p (b x) -> p b x", b=BS)[:, :, off : off + nq * C]
                        .rearrange("p b (q c) -> p q b c", q=nq)
                    )
                    lo = wklo_t[:, k * PC_N + m : k * PC_N + m + 1]
                    hi = wkhi_t[:, k * PC_N + m : k * PC_N + m + 1]
                    psv = ps[:, :n].rearrange("p (q b c) -> p q b c", q=nq, b=BS)
                    if m % 2 == 0:
                        nc.vector.tensor_scalar(dst, psv, lo, hi, Alu.max, Alu.min)
                    else:
                        ev = scr.tile([128, 448], f16, tag="ev", bufs=3, name=f"ev{pi}{k}{rs[0]}{m}")
                        evs = ev[:, :n]
                        nc.scalar.activation(evs, ps[:, :n], Act.Identity)
                        nc.vector.tensor_scalar(
                            dst,
                            evs.rearrange("p (q b c) -> p q b c", q=nq, b=BS),
                            lo, hi, Alu.max, Alu.min,
                        )

            # ---- phase 1: GRU (fused gates) ----
            with (
                tc.tile_pool(name="p1", bufs=1) as p1,
                tc.tile_pool(name="psG", bufs=2, space="PSUM") as psG,
                tc.tile_pool(name="psH", bufs=2, space="PSUM") as psH,
            ):
                wih_b = p1.tile([128, PC_N * 768], f16, name="wih_b")
                whh_b = p1.tile([128, HC_N * 768], f16, name="whh_b")
                enc_b = p1.tile([128, R * PC_N * BC], f16, name="enc_b")
                # wih is m-major [m, pc, 128]; split across both HWDGE queues
                # ordered by first use: gin (m4/m5), then t0 (m0, m2), t1 (m1, m3)
                for m in (4, 0, 2):
                    nc.sync.dma_start(
                        out=wih_b[:, m * P : (m + 1) * P], in_=wih[:, m * P : (m + 1) * P]
                    )
                for r in (0, 1):
                    sl = slice(r * PC_N * BC, (r + 1) * PC_N * BC)
                    nc.scalar.dma_start(out=enc_b[:, sl], in_=encT[:, sl])
                for m in (5, 1, 3):
                    nc.scalar.dma_start(
                        out=wih_b[:, m * P : (m + 1) * P], in_=wih[:, m * P : (m + 1) * P]
                    )
                nc.sync.dma_start(out=whh_b, in_=whh[:, :])
                for r in range(2, R):
                    sl = slice(r * PC_N * BC, (r + 1) * PC_N * BC)
                    nc.scalar.dma_start(out=enc_b[:, sl], in_=encT[:, sl])
                nc.scalar.dma_start(out=encB_b[:, :hw_], in_=encB[:, :hw_])
                encv = enc_b.rearrange("p (r pc x) -> p r pc x", r=R, pc=PC_N)

                def wih_s(pc, m):
                    return wih_b[:, m * P + pc * 128 : m * P + (pc + 1) * 128]

                def whh_s(hc, m):
                    return whh_b[:, hc * 768 + m * 128 : hc * 768 + (m + 1) * 128]

                gin = [p1.tile([128, R * BC], f16, name=f"gin{t}") for t in range(2)]

                def emit_gin_chunk(t, ch):
                    # gi for the n gate, steps 2ch and 2ch+1 (448 cols)
                    ps = psPP.tile([128, 512], f32, tag="pp", name=f"gin{t}_{ch}")
                    for pc in range(PC_N):
                        nc.tensor.matmul(
                            ps[:, : 2 * BC],
                            wih_s(pc, 4 + t),
                            encv[:, 2 * ch : 2 * ch + 2, pc : pc + 1, :],
                            start=(pc == 0),
                            stop=(pc == PC_N - 1),
                        )
                    nc.vector.tensor_scalar(
                        gin[t][:, 2 * ch * BC : (2 * ch + 2) * BC],
                        ps[:, : 2 * BC],
                        bin_t[:, t : t + 1], 0.0, Alu.add, Alu.add,
                    )

                load_wk(4)
                load_wk(3)
                load_wk(2)
                load_wk(0)
                load_wk(1)

                def emit_gru_step(r):
                    hprev = [h0, h0] if r == 0 else [
                        ctx[t][:, (r - 1) * BC : r * BC] for t in range(2)
                    ]
                    gps = []
                    hps = []
                    for t in range(2):
                        ps = psG.tile([128, 448], f32, tag="g", name=f"g{r}_{t}")
                        for half, m in ((0, t), (1, 2 + t)):   # r gate, z gate
                            sl = ps[:, half * BC : (half + 1) * BC]
                            for pc in range(PC_N):
                                nc.tensor.matmul(
                                    sl, wih_s(pc, m),
                                    enc_b[:, (r * PC_N + pc) * BC : (r * PC_N + pc + 1) * BC],
                                    start=(pc == 0), stop=False,
                                )
                            for hc in range(HC_N):
                                nc.tensor.matmul(
                                    sl, whh_s(hc, m), hprev[hc],
                                    start=False, stop=(hc == HC_N - 1),
                                )
                        gps.append(ps)
                        ph = psH.tile([128, BC], f32, tag="h", name=f"h{r}_{t}")
                        for hc in range(HC_N):
                            nc.tensor.matmul(
                                ph, whh_s(hc, 4 + t), hprev[hc],
                                start=(hc == 0), stop=(hc == HC_N - 1),
                            )
                        hps.append(ph)
                    for t in range(2):
                        gr = gps[t][:, 0:BC]
                        gz = gps[t][:, BC : 2 * BC]
                        rt = scr.tile([128, BC], f16, tag="rt", bufs=2, name=f"rt{r}{t}")
                        nc.scalar.activation(rt, gr, Act.Sigmoid, bias=brz_t[:, t : t + 1])
                        zt = scr.tile([128, BC], f16, tag="zt", bufs=2, name=f"zt{r}{t}")
                        nc.scalar.activation(zt, gz, Act.Sigmoid, bias=brz_t[:, 2 + t : 3 + t])
                        z1 = scr.tile([128, BC], f16, tag="z1", bufs=2, name=f"z1{r}{t}")
                        nc.scalar.activation(
                            z1, gz, Act.Sigmoid, bias=nbrz_t[:, t : t + 1], scale=-1.0
                        )
                        hns = scr.tile([128, BC], f16, tag="hns", bufs=2, name=f"hns{r}{t}")
                        nc.vector.tensor_scalar(
                            hns, hps[t], bhn_t[:, t : t + 1], 0.0, Alu.add, Alu.add
                        )
                        tV = scr.tile([128, BC], f16, tag="tV", bufs=2, name=f"tV{r}{t}")
                        nc.vector.tensor_tensor(tV, hns, rt, op=Alu.mult)
                        tW = scr.tile([128, BC], f16, tag="tW", bufs=2, name=f"tW{r}{t}")
                        nc.vector.tensor_tensor(
                            tW, tV, gin[t][:, r * BC : (r + 1) * BC], op=Alu.add
                        )
                        sv = scr.tile([128, BC], f16, tag="sv", bufs=2, name=f"sv{r}{t}")
                        nc.scalar.activation(sv, tW, Act.Sigmoid, scale=2.0)
                        a_ = scr.tile([128, BC], f16, tag="a_", bufs=2, name=f"a{r}{t}")
                        nc.vector.tensor_tensor(a_, sv, z1, op=Alu.mult)
                        b2 = scr.tile([128, BC], f16, tag="b2", bufs=2, name=f"b{r}{t}")
                        nc.vector.tensor_tensor(b2, zt, hprev[t], op=Alu.mult)
                        nc.vector.scalar_tensor_tensor(
                            ctx[t][:, r * BC : (r + 1) * BC],
                            a_, 2.0, b2, op0=Alu.mult, op1=Alu.add,
                        )

                emit_gin_chunk(0, 0)
                emit_gin_chunk(1, 0)
                emit_gru_step(0)
                emit_gin_chunk(0, 1)
                emit_gin_chunk(1, 1)
                emit_gru_step(1)
                emit_gin_chunk(0, 2)
                emit_preds_chunk(0, 4, [0, 1])
                emit_gru_step(2)
                emit_gin_chunk(1, 2)
                emit_preds_chunk(0, 3, [0, 1])
                emit_preds_chunk(0, 3, [2])
                emit_gru_step(3)
                emit_preds_chunk(1, 1, [0, 1])
                emit_gru_step(4)
                emit_preds_chunk(0, 2, [0, 1])
                emit_preds_chunk(0, 2, [2, 3])
                emit_preds_chunk(1, 0, [0, 1])
                emit_gru_step(5)
                emit_preds_chunk(1, 0, [2, 3])
                emit_preds_chunk(1, 1, [2, 3])
                emit_preds_chunk(1, 1, [4])
                emit_preds_chunk(1, 0, [4, 5])

            # ---- phase 3: dots + loss ----
            with (
                tc.tile_pool(name="p3", bufs=1) as p3,
                tc.tile_pool(name="psDP", bufs=3, space="PSUM") as psDP,
            ):
                cnt_t = [
                    p3.tile([PR[pi], BS * IJ], f16, name=f"cnt{pi}") for pi in range(2)
                ]
                corr_t = [
                    p3.tile([PR[pi], BS * IJ], f16, name=f"corr{pi}") for pi in range(2)
                ]
                D_t = [
                    p3.tile([PR[pi], BS * IJ], f16, name=f"D{pi}") for pi in range(2)
                ]
                outT = [
                    p3.tile([PR[pi], 2 * IJ], f32, name=f"outT{pi}") for pi in range(2)
                ]
                for pi in range(2):
                    nc.sync.dma_start(out=corr_t[pi], in_=corr_d[pi][:, :])
                for pi in range(2):
                    nc.sync.dma_start(out=cnt_t[pi], in_=cnt_d[pi][:, :])

                def emit_dots(pi, bb):
                    rows = PR[pi]
                    ps = psDP.tile([rows, 2 * IJ], f32, tag="dp", name=f"dp{pi}_{bb}")
                    for half in range(2):
                        b = 2 * bb + half
                        for pc in range(PC_N):
                            nc.tensor.matmul(
                                ps[:, half * IJ : (half + 1) * IJ],
                                predsT[pi][pc][:, b * rows : (b + 1) * rows],
                                encB_b[:, pc * BS * IJ + b * IJ : pc * BS * IJ + (b + 1) * IJ],
                                start=(pc == 0),
                                stop=(pc == PC_N - 1),
                            )
                    csl = slice(2 * bb * IJ, (2 * bb + 2) * IJ)
                    nc.vector.tensor_tensor(
                        D_t[pi][:, csl], ps, corr_t[pi][:, csl], op=Alu.add
                    )

                PG = BS // 2   # 16 groups per post part

                def emit_post(pi, h):
                    rows = PR[pi]
                    c0 = h * PG * IJ
                    ob = h * IJ
                    Dp = D_t[pi][:, c0 : c0 + PG * IJ]
                    Dv = Dp.rearrange("p (g j) -> p g j", j=IJ)
                    mxg = outT[pi][:, ob + 32 : ob + 48]
                    nc.vector.tensor_reduce(mxg, Dv, axis=AxX, op=Alu.max)
                    mxp = outT[pi][:, ob + 48 : ob + 49]
                    nc.vector.tensor_reduce(mxp, mxg, axis=AxX, op=Alu.max)
                    nmx = scr.tile([rows, 1], f32, tag=f"nmx{pi}", bufs=2, name=f"nmx{pi}{h}")
                    nc.vector.tensor_scalar(nmx, mxp, -1.0, 0.0, Alu.mult, Alu.add)
                    B2 = p3.tile([rows, PG * IJ], f32, tag=f"B2{pi}", bufs=2, name=f"B2{pi}{h}")
                    nc.scalar.activation(B2, Dp, Act.Exp, bias=nmx[:, 0:1])
                    nc.vector.tensor_tensor(
                        B2, B2, cnt_t[pi][:, c0 : c0 + PG * IJ], op=Alu.mult
                    )
                    se = outT[pi][:, ob : ob + 16]
                    nc.vector.tensor_reduce(
                        se, B2.rearrange("p (g j) -> p g j", j=IJ), axis=AxX, op=Alu.add
                    )
                    P2 = p3.tile([rows, PG * IJ], f16, tag=f"P2{pi}", bufs=2, name=f"P2{pi}{h}")
                    nc.vector.tensor_tensor(
                        P2.rearrange("p (g j) -> p g j", j=IJ),
                        Dv,
                        posm_t[pi].unsqueeze(1).broadcast_to([rows, PG, IJ]),
                        op=Alu.mult,
                    )
                    pos = outT[pi][:, ob + 16 : ob + 32]
                    nc.vector.tensor_reduce(
                        pos, P2.rearrange("p (g j) -> p g j", j=IJ), axis=AxX, op=Alu.add
                    )

                for bb in range(BS // 2):
                    emit_dots(0, bb)
                emit_post(0, 0)
                for bb in range(BS // 2):
                    emit_dots(1, bb)
                emit_post(0, 1)
                nc.sync.dma_start(out=out_d[0][:, :], in_=outT[0])
                emit_post(1, 0)
                emit_post(1, 1)
                nc.sync.dma_start(out=out_d[1][:, :], in_=outT[1])
            psPP.release()
            wkp.release()
            scr.release()

    nc.finalize()
    return nc


def _prep_inputs(encodings, hidden, W_ih, W_hh, b_ih, b_hh, Wk_w, Wk_b,
                 neg_rows, neg_cols):
    """Host-side reformat of the full inputs into per-core DMA-clean arrays."""
    f16 = np.float16
    enc = np.ascontiguousarray(encodings, dtype=np.float32)
    e6 = enc.reshape(NCORE, BS, C, C, PC_N, 128)  # (core, b, i, c, pc, pp)
    # GRU layout (r-major): [core, pp, r, pc, b*7+c]
    encT = np.ascontiguousarray(
        e6[:, :, :R].transpose(0, 5, 2, 4, 1, 3)   # (core, pp, r, pc, b, c)
    ).reshape(NCORE, 128, R * PC_N * BC).astype(f16)
    # dots layout: [core, pp, pc, b*49 + i*7 + c]
    encB = np.ascontiguousarray(
        e6.transpose(0, 5, 4, 1, 2, 3)   # (core, pp, pc, b, i, c)
    ).reshape(NCORE, 128, PC_N * BS * IJ).astype(f16)

    # m-major: [pp, m, pc, 128] so per-gate slices stream independently
    wih = np.ascontiguousarray(
        W_ih.reshape(6, 128, PC_N, 128).transpose(3, 0, 2, 1), dtype=np.float32
    ).reshape(128, 6 * P).astype(f16)
    whh = np.ascontiguousarray(
        W_hh.T.reshape(HC_N, 128, 768).transpose(1, 0, 2), dtype=np.float32
    ).reshape(128, HC_N * 768).astype(f16)
    wkh = np.ascontiguousarray(
        Wk_w.transpose(0, 2, 1).reshape(K, HC_N, 128, P).transpose(0, 2, 1, 3),
        dtype=np.float32,
    ).reshape(K, 128, HC_N * P).astype(f16)

    # H' = h + 1 bias folds
    rs = W_hh.astype(np.float32).sum(axis=1)              # [768]
    bsum = (b_ih + b_hh).astype(np.float32) - rs
    brz = np.ascontiguousarray(bsum[:512].reshape(4, 128).T)
    nbrz = np.ascontiguousarray((-bsum[256:512]).reshape(2, 128).T)
    bhn = np.ascontiguousarray(
        (b_hh.astype(np.float32) - rs)[512:].reshape(2, 128).T
    )
    bin_ = np.ascontiguousarray(b_ih[512:].astype(np.float32).reshape(2, 128).T)

    bias_k = Wk_b.astype(np.float32) - Wk_w.astype(np.float32).sum(axis=2)  # [K, P]
    wkbT = np.ascontiguousarray(
        bias_k.reshape(K, PC_N, 128).transpose(2, 0, 1)
    ).reshape(128, K * PC_N)
    wklo = -1.0 - wkbT
    wkhi = 1.0 - wkbT
    corr_k = np.einsum("kp,bijp->kbij", bias_k, enc, optimize=True).reshape(K, B, IJ)

    # negatives -> multiplicity counts over the 49 cells, plus the positive
    neg_idx = (neg_rows.astype(np.int64) * 7 + neg_cols.astype(np.int64))
    cnts = np.zeros((B, K, R, C, IJ), dtype=np.float32)
    np.add.at(
        cnts.reshape(B * K * R * C, IJ),
        (
            np.repeat(np.arange(B * K * R * C, dtype=np.int64), S - 1),
            neg_idx.reshape(-1),
        ),
        1.0,
    )
    cvec = np.arange(C)
    for k in range(K):
        for r in range(R):
            cnts[:, k, r, cvec, r * 7 + cvec] += 1.0   # the positive

    corr_dev, cnt_dev, posm_dev = [], [], []
    for pi, lst in enumerate(PASS_PAIRS):
        rows = PR[pi]
        cd = np.empty((NCORE, rows, BS, IJ), dtype=np.float32)
        nd = np.empty((NCORE, rows, BS, IJ), dtype=np.float32)
        pm = np.zeros((rows, IJ), dtype=np.float32)
        for qi, (k, r) in enumerate(lst):
            for c in range(C):
                row = qi * C + c
                cd[:, row] = corr_k[k].reshape(NCORE, BS, IJ)
                nd[:, row] = cnts[:, k, r, c].reshape(NCORE, BS, IJ)
                pm[row, r * 7 + c] = 1.0
        cd = cd + np.float32(MASK) * (nd == 0)
        corr_dev.append(cd.reshape(NCORE, rows, BS * IJ).astype(f16))
        cnt_dev.append(nd.reshape(NCORE, rows, BS * IJ).astype(f16))
        posm_dev.append(pm.astype(f16))

    in_maps = []
    for core in range(NCORE):
        in_maps.append(
            {
                "encT": encT[core],
                "encB": encB[core],
                "wih": wih,
                "whh": whh,
                "wk": wkh,
                "brz": brz,
                "nbrz": nbrz,
                "bhn": bhn,
                "bin": bin_,
                "wklo": wklo,
                "wkhi": wkhi,
                "corr0": corr_dev[0][core],
                "corr1": corr_dev[1][core],
                "cnt0": cnt_dev[0][core],
                "cnt1": cnt_dev[1][core],
                "posm0": posm_dev[0],
                "posm1": posm_dev[1],
            }
        )
    return in_maps


def _get_program():
    if "nc" not in _CACHE:
        _CACHE["nc"] = _build_program()
    return _CACHE["nc"]


def run_on_device(in_maps, trace=False, tmpdir=None):
    from concourse.bass_utils import run_bass_kernel_spmd

    nc = _get_program()
    return run_bass_kernel_spmd(
        nc, in_maps, list(range(NCORE)), trace=trace, tmpdir=tmpdir
    )


def kernel(**inputs):
    in_maps = _prep_inputs(**inputs)
    res = run_on_device(in_maps)
    loss_sum = 0.0
    corr_sum = 0.0
    for core in range(NCORE):
        for pi in range(2):
            o = np.asarray(res.results[core][f"out{pi}"], dtype=np.float64)
            o = o.reshape(PR[pi], 2, IJ)
            se = o[:, :, 0:16]
            pos = o[:, :, 16:32]
            mxg = o[:, :, 32:48]
            mxp = o[:, :, 48:49]
            loss_sum += np.sum(np.log(se) + mxp - pos)
            corr_sum += np.sum(pos >= mxg)
    loss = np.float32(loss_sum / N_PREDS)
    acc = np.float32(corr_sum / N_PREDS)
    return loss, acc


# revision 31
# speedup vs baseline: 1.1639x; 1.1639x over previous
"""Trainium2 Bass kernel for nn_CDC_62646392980082 (GRU-CPC loss_fn), v2.

Contract: kernel(**inputs) takes the FULL unsharded inputs (numpy) and
returns the FULL output (loss, acc) exactly like the jax reference.

Strategy (8 NeuronCores, data-parallel over batch B=256 -> 32/core):
  - GRU gates fused in PSUM: gi (x@W_ih) and gh (h@W_hh) accumulate into
    the same PSUM bank per step; sigmoid reads PSUM directly on the
    scalar engine (no gi copies / adds on the DVE).
  - H' = h+1 reparameterization: h' = (n+1)(1-z) + z*h' with n+1 =
    2*sigmoid(2x); rank-1 corrections folded into biases host-side.
    Avoids tanh table swaps and one DVE op per gate step.
  - preds split into two passes by r: pass0 = {k2:r0-3, k3:r0-2,
    k4:r0-1} (63 rows), pass1 = {k0:r0-5, k1:r0-4} (77 rows), so
    pass0's dots can overlap the preds tail.
  - clip alternates engines per p-chunk: even chunks clip directly from
    PSUM on the DVE; odd chunks evacuate via scalar-engine Identity and
    clip fp16->fp16 on the DVE fast path.
  - negatives folded host-side into multiplicity counts; the cnt==0
    mask (-60000) is folded into the corr tensor; softmax shift uses a
    per-partition-row max so exp's bias port applies it for free.
"""

import sys

if "/opt/trn_rl_repo" not in sys.path:
    sys.path.insert(0, "/opt/trn_rl_repo")

import numpy as np

B, K, R, C, P, H, S = 256, 5, 6, 7, 1280, 256, 64
NCORE = 8
BS = B // NCORE            # 32 images per core
BC = BS * C                # 224 (b, c) columns
PC_N = P // 128            # 10 p-chunks
HC_N = H // 128            # 2 h-chunks
IJ = 49                    # 7x7 cells

# pass structure: pairs (k, r) grouped so pass0 finishes by GRU step 3
PASS_PAIRS = [
    [(2, 0), (2, 1), (2, 2), (2, 3), (3, 0), (3, 1), (3, 2), (4, 0), (4, 1)],
    [(0, 0), (0, 1), (0, 2), (0, 3), (0, 4), (0, 5),
     (1, 0), (1, 1), (1, 2), (1, 3), (1, 4)],
]
PR = [len(PASS_PAIRS[0]) * C, len(PASS_PAIRS[1]) * C]   # 63, 77 rows
ROW_OFF = {}
for _pi, _lst in enumerate(PASS_PAIRS):
    for _qi, _kr in enumerate(_lst):
        ROW_OFF[_kr] = (_pi, _qi * C)

# preds chunks: (pass, k, [r...]) with adjacent r, emitted after step max(r)
CHUNKS = [
    (0, 4, [0, 1]), (0, 3, [0, 1]),            # ready after step 1
    (0, 3, [2]), (1, 1, [0, 1]),               # after step 2
    (0, 2, [0, 1]), (0, 2, [2, 3]),
    (1, 0, [0, 1]), (1, 0, [2, 3]), (1, 1, [2, 3]),   # after step 3
    (1, 1, [4]),                               # after step 4
    (1, 0, [4, 5]),                            # after step 5
]
N_PREDS = 20 * B * C       # 35840 global predictions
MASK = np.float32(-60000.0)

_CACHE = {}


def _build_program():
    import concourse.bacc as bacc
    import concourse.mybir as mybir
    from concourse.tile import TileContext

    f32 = mybir.dt.float32
    f16 = mybir.dt.float16
    Alu = mybir.AluOpType
    Act = mybir.ActivationFunctionType
    AxX = mybir.AxisListType.X

    nc = bacc.Bacc()
    dp = nc.declare_dram_parameter
    encT = dp("encT", [128, R * PC_N * BC], f16, isOutput=False)   # r-major
    encB = dp("encB", [128, PC_N * BS * IJ], f16, isOutput=False)
    wih = dp("wih", [128, PC_N * 768], f16, isOutput=False)
    whh = dp("whh", [128, HC_N * 768], f16, isOutput=False)
    wk = dp("wk", [K, 128, HC_N * P], f16, isOutput=False)
    # [brz 4][nbrz 2][bhn 2][bin 2][wklo 50][wkhi 50]
    smA = dp("smA", [128, 110], f32, isOutput=False)
    corr0 = dp("corr0", [PR[0], BS * IJ], f16, isOutput=False)  # corr - mask
    corr1 = dp("corr1", [PR[1], BS * IJ], f16, isOutput=False)
    cnt0 = dp("cnt0", [PR[0], BS * IJ], f16, isOutput=False)
    cnt1 = dp("cnt1", [PR[1], BS * IJ], f16, isOutput=False)
    posmAB = dp("posmAB", [PR[1], 2 * IJ], f16, isOutput=False)
    # per (pass, half): [se 16][pos 16][mxg 16][mxp 1] in 49-col blocks
    out0 = dp("out0", [PR[0], 2 * IJ], f32, isOutput=True)
    out1 = dp("out1", [PR[1], 2 * IJ], f32, isOutput=True)
    corr_d = [corr0, corr1]
    cnt_d = [cnt0, cnt1]
    out_d = [out0, out1]

    with TileContext(nc, pool_alloc_mode="queue") as tc:
        with tc.tile_pool(name="pers", bufs=1) as pers:
            # ---- persistent small loads (combined to minimize dispatches) ----
            smA_t = pers.tile([128, 110], f32, name="smA")
            posm_b = pers.tile([PR[1], 2 * IJ], f16, name="posmAB")
            brz_t = smA_t[:, 0:4]
            nbrz_t = smA_t[:, 4:6]
            bhn_t = smA_t[:, 6:8]
            bin_t = smA_t[:, 8:10]
            wklo_t = smA_t[:, 10:60]
            wkhi_t = smA_t[:, 60:110]
            posm_t = [posm_b[: PR[0], 0:IJ], posm_b[:, IJ : 2 * IJ]]

            # dots-phase encodings: loaded early on the gpsimd/scalar queues
            encB_b = pers.tile([128, PC_N * BS * IJ], f16, name="encB_b")
            hw_ = PC_N * BS * IJ // 2
            nc.gpsimd.dma_start(out=encB_b[:, hw_:], in_=encB[:, hw_:])

            # GRU context (H' = h+1), per h-chunk: [128, r*224]
            ctx = [pers.tile([128, R * BC], f16, name=f"ctx{t}") for t in range(2)]
            h0 = pers.tile([128, BC], f16)
            nc.vector.memset(h0, 1.0)

            predsT = [
                [
                    pers.tile([128, BS * PR[pi]], f16, name=f"pt{pi}_{m}")
                    for m in range(PC_N)
                ]
                for pi in range(2)
            ]
            corr_t = [
                pers.tile([PR[pi], BS * IJ], f16, name=f"corr{pi}") for pi in range(2)
            ]

            # scratch pools
            scr = tc.alloc_tile_pool(name="scr", bufs=1)
            wkp = tc.alloc_tile_pool(name="wkp", bufs=1)
            psPP = tc.alloc_tile_pool(name="psPP", bufs=3, space="PSUM")

            wk_t = {}

            def load_wk(k):
                t = wkp.tile([128, HC_N * P], f16, tag="wk", bufs=3, name=f"wk{k}")
                nc.sync.dma_start(out=t, in_=wk[k, :, :])
                wk_t[k] = t

            # ---------- preds emission ----------
            def emit_preds_chunk(pi, k, rs):
                nq = len(rs)
                n = nq * BC
                for m in range(PC_N):
                    ps = psPP.tile([128, 512], f32, tag="pp", name=f"pp{pi}_{k}_{rs[0]}_{m}")
                    for hc in range(HC_N):
                        nc.tensor.matmul(
                            ps[:, :n],
                            wk_t[k][:, hc * P + m * 128 : hc * P + (m + 1) * 128],
                            ctx[hc][:, rs[0] * BC : (rs[0] + nq) * BC],
                            start=(hc == 0),
                            stop=(hc == HC_N - 1),
                        )
                    off = ROW_OFF[(k, rs[0])][1]
                    dst = (
                        predsT[pi][m]
                        .rearrange("p (b x) -> p b x", b=BS)[:, :, off : off + nq * C]
                        .rearrange("p b (q c) -> p q b c", q=nq)
                    )
                    lo = wklo_t[:, k * PC_N + m : k * PC_N + m + 1]
                    hi = wkhi_t[:, k * PC_N + m : k * PC_N + m + 1]
                    psv = ps[:, :n].rearrange("p (q b c) -> p q b c", q=nq, b=BS)
                    if m % 2 == 0:
                        nc.vector.tensor_scalar(dst, psv, lo, hi, Alu.max, Alu.min)
                    else:
                        ev = scr.tile([128, 448], f16, tag="ev", bufs=3, name=f"ev{pi}{k}{rs[0]}{m}")
                        evs = ev[:, :n]
                        nc.scalar.activation(evs, ps[:, :n], Act.Identity)
                        nc.vector.tensor_scalar(
                            dst,
                            evs.rearrange("p (q b c) -> p q b c", q=nq, b=BS),
                            lo, hi, Alu.max, Alu.min,
                        )

            # ---- phase 1: GRU (fused gates) ----
            with (
                tc.tile_pool(name="p1", bufs=1) as p1,
                tc.tile_pool(name="psG", bufs=2, space="PSUM") as psG,
                tc.tile_pool(name="psH", bufs=2, space="PSUM") as psH,
            ):
                wih_b = p1.tile([128, PC_N * 768], f16, name="wih_b")
                whh_b = p1.tile([128, HC_N * 768], f16, name="whh_b")
                enc_b = p1.tile([128, R * PC_N * BC], f16, name="enc_b")
                # wih is m-major [m, pc, 128]; split across both HWDGE queues
                # ordered by first use: gin (m4/m5), then t0 (m0, m2), t1 (m1, m3)
                nc.sync.dma_start(out=wih_b[:, 4 * P : 5 * P], in_=wih[:, 4 * P : 5 * P])
                nc.sync.dma_start(out=smA_t, in_=smA[:, :])
                nc.sync.dma_start(out=posm_b, in_=posmAB[:, :])
                for r in (0, 1):
                    sl = slice(r * PC_N * BC, (r + 1) * PC_N * BC)
                    nc.scalar.dma_start(out=enc_b[:, sl], in_=encT[:, sl])
                for m in (0, 2):
                    nc.sync.dma_start(
                        out=wih_b[:, m * P : (m + 1) * P], in_=wih[:, m * P : (m + 1) * P]
                    )
                for m in (5, 1, 3):
                    nc.scalar.dma_start(
                        out=wih_b[:, m * P : (m + 1) * P], in_=wih[:, m * P : (m + 1) * P]
                    )
                nc.sync.dma_start(out=whh_b, in_=whh[:, :])
                for r in range(2, R):
                    sl = slice(r * PC_N * BC, (r + 1) * PC_N * BC)
                    nc.scalar.dma_start(out=enc_b[:, sl], in_=encT[:, sl])
                nc.scalar.dma_start(out=encB_b[:, :hw_], in_=encB[:, :hw_])
                encv = enc_b.rearrange("p (r pc x) -> p r pc x", r=R, pc=PC_N)

                def wih_s(pc, m):
                    return wih_b[:, m * P + pc * 128 : m * P + (pc + 1) * 128]

                def whh_s(hc, m):
                    return whh_b[:, hc * 768 + m * 128 : hc * 768 + (m + 1) * 128]

                gin = [p1.tile([128, R * BC], f16, name=f"gin{t}") for t in range(2)]

                def emit_gin_chunk(t, ch):
                    # gi for the n gate, steps 2ch and 2ch+1 (448 cols)
                    ps = psPP.tile([128, 512], f32, tag="pp", name=f"gin{t}_{ch}")
                    for pc in range(PC_N):
                        nc.tensor.matmul(
                            ps[:, : 2 * BC],
                            wih_s(pc, 4 + t),
                            encv[:, 2 * ch : 2 * ch + 2, pc : pc + 1, :],
                            start=(pc == 0),
                            stop=(pc == PC_N - 1),
                        )
                    nc.vector.tensor_scalar(
                        gin[t][:, 2 * ch * BC : (2 * ch + 2) * BC],
                        ps[:, : 2 * BC],
                        bin_t[:, t : t + 1], 0.0, Alu.add, Alu.add,
                    )

                load_wk(4)
                load_wk(3)
                load_wk(2)
                load_wk(1)
                load_wk(0)
                for pi in range(2):
                    nc.sync.dma_start(out=corr_t[pi], in_=corr_d[pi][:, :])

                def emit_gru_step(r):
                    hprev = [h0, h0] if r == 0 else [
                        ctx[t][:, (r - 1) * BC : r * BC] for t in range(2)
                    ]
                    gps = []
                    hps = []
                    for t in range(2):
                        ps = psG.tile([128, 448], f32, tag="g", name=f"g{r}_{t}")
                        for half, m in ((0, t), (1, 2 + t)):   # r gate, z gate
                            sl = ps[:, half * BC : (half + 1) * BC]
                            for pc in range(PC_N):
                                nc.tensor.matmul(
                                    sl, wih_s(pc, m),
                                    enc_b[:, (r * PC_N + pc) * BC : (r * PC_N + pc + 1) * BC],
                                    start=(pc == 0), stop=False,
                                )
                            for hc in range(HC_N):
                                nc.tensor.matmul(
                                    sl, whh_s(hc, m), hprev[hc],
                                    start=False, stop=(hc == HC_N - 1),
                                )
                        gps.append(ps)
                        ph = psH.tile([128, BC], f32, tag="h", name=f"h{r}_{t}")
                        for hc in range(HC_N):
                            nc.tensor.matmul(
                                ph, whh_s(hc, 4 + t), hprev[hc],
                                start=(hc == 0), stop=(hc == HC_N - 1),
                            )
                        hps.append(ph)
                    for t in range(2):
                        gr = gps[t][:, 0:BC]
                        gz = gps[t][:, BC : 2 * BC]
                        rt = scr.tile([128, BC], f16, tag="rt", bufs=2, name=f"rt{r}{t}")
                        nc.scalar.activation(rt, gr, Act.Sigmoid, bias=brz_t[:, t : t + 1])
                        zt = scr.tile([128, BC], f16, tag="zt", bufs=2, name=f"zt{r}{t}")
                        nc.scalar.activation(zt, gz, Act.Sigmoid, bias=brz_t[:, 2 + t : 3 + t])
                        z1 = scr.tile([128, BC], f16, tag="z1", bufs=2, name=f"z1{r}{t}")
                        nc.scalar.activation(
                            z1, gz, Act.Sigmoid, bias=nbrz_t[:, t : t + 1], scale=-1.0
                        )
                        tV = scr.tile([128, BC], f16, tag="tV", bufs=2, name=f"tV{r}{t}")
                        nc.vector.scalar_tensor_tensor(
                            tV, hps[t], bhn_t[:, t : t + 1], rt, op0=Alu.add, op1=Alu.mult
                        )
                        tW = scr.tile([128, BC], f16, tag="tW", bufs=2, name=f"tW{r}{t}")
                        nc.vector.tensor_tensor(
                            tW, tV, gin[t][:, r * BC : (r + 1) * BC], op=Alu.add
                        )
                        sv = scr.tile([128, BC], f16, tag="sv", bufs=2, name=f"sv{r}{t}")
                        nc.scalar.activation(sv, tW, Act.Sigmoid, scale=2.0)
                        a_ = scr.tile([128, BC], f16, tag="a_", bufs=2, name=f"a{r}{t}")
                        nc.vector.tensor_tensor(a_, sv, z1, op=Alu.mult)
                        b2 = scr.tile([128, BC], f16, tag="b2", bufs=2, name=f"b{r}{t}")
                        nc.vector.tensor_tensor(b2, zt, hprev[t], op=Alu.mult)
                        nc.vector.scalar_tensor_tensor(
                            ctx[t][:, r * BC : (r + 1) * BC],
                            a_, 2.0, b2, op0=Alu.mult, op1=Alu.add,
                        )

                emit_gin_chunk(0, 0)
                emit_gin_chunk(1, 0)
                emit_gru_step(0)
                emit_gin_chunk(0, 1)
                emit_gin_chunk(1, 1)
                emit_gru_step(1)
                emit_gin_chunk(0, 2)
                emit_preds_chunk(0, 4, [0, 1])
                emit_gru_step(2)
                emit_gin_chunk(1, 2)
                emit_preds_chunk(0, 3, [0, 1])
                emit_gru_step(3)
                emit_preds_chunk(0, 3, [2])
                emit_preds_chunk(0, 2, [0, 1])
                emit_gru_step(4)
                emit_preds_chunk(0, 2, [2, 3])
                emit_gru_step(5)

            # ---- phase 3: pass-1 preds + dots + loss ----
            with (
                tc.tile_pool(name="p3", bufs=1) as p3,
                tc.tile_pool(name="psDP", bufs=3, space="PSUM") as psDP,
            ):
                cnt_t = [
                    p3.tile([PR[pi], BS * IJ], f16, name=f"cnt{pi}") for pi in range(2)
                ]
                D_t = [
                    p3.tile([PR[pi], BS * IJ], f16, name=f"D{pi}") for pi in range(2)
                ]
                outT = [
                    p3.tile([PR[pi], 2 * IJ], f32, name=f"outT{pi}") for pi in range(2)
                ]
                for pi in range(2):
                    nc.sync.dma_start(out=cnt_t[pi], in_=cnt_d[pi][:, :])

                def emit_dots(pi, bb):
                    rows = PR[pi]
                    ps = psDP.tile([rows, 2 * IJ], f32, tag="dp", name=f"dp{pi}_{bb}")
                    for half in range(2):
                        b = 2 * bb + half
                        for pc in range(PC_N):
                            nc.tensor.matmul(
                                ps[:, half * IJ : (half + 1) * IJ],
                                predsT[pi][pc][:, b * rows : (b + 1) * rows],
                                encB_b[:, pc * BS * IJ + b * IJ : pc * BS * IJ + (b + 1) * IJ],
                                start=(pc == 0),
                                stop=(pc == PC_N - 1),
                            )
                    csl = slice(2 * bb * IJ, (2 * bb + 2) * IJ)
                    nc.vector.tensor_tensor(
                        D_t[pi][:, csl], ps, corr_t[pi][:, csl], op=Alu.add
                    )

                PG = BS // 2   # 16 groups per post part

                def emit_post(pi, h):
                    rows = PR[pi]
                    c0 = h * PG * IJ
                    ob = h * IJ
                    Dp = D_t[pi][:, c0 : c0 + PG * IJ]
                    Dv = Dp.rearrange("p (g j) -> p g j", j=IJ)
                    mxg = outT[pi][:, ob + 32 : ob + 48]
                    nc.vector.tensor_reduce(mxg, Dv, axis=AxX, op=Alu.max)
                    mxp = outT[pi][:, ob + 48 : ob + 49]
                    nc.vector.tensor_reduce(mxp, mxg, axis=AxX, op=Alu.max)
                    nmx = scr.tile([rows, 1], f32, tag=f"nmx{pi}", bufs=2, name=f"nmx{pi}{h}")
                    nc.vector.tensor_scalar(nmx, mxp, -1.0, 0.0, Alu.mult, Alu.add)
                    B2 = p3.tile([rows, PG * IJ], f32, tag=f"B2{pi}", bufs=2, name=f"B2{pi}{h}")
                    nc.scalar.activation(B2, Dp, Act.Exp, bias=nmx[:, 0:1])
                    nc.vector.tensor_tensor(
                        B2, B2, cnt_t[pi][:, c0 : c0 + PG * IJ], op=Alu.mult
                    )
                    se = outT[pi][:, ob : ob + 16]
                    nc.vector.tensor_reduce(
                        se, B2.rearrange("p (g j) -> p g j", j=IJ), axis=AxX, op=Alu.add
                    )
                    P2 = p3.tile([rows, PG * IJ], f16, tag=f"P2{pi}", bufs=2, name=f"P2{pi}{h}")
                    nc.vector.tensor_tensor(
                        P2.rearrange("p (g j) -> p g j", j=IJ),
                        Dv,
                        posm_t[pi].unsqueeze(1).broadcast_to([rows, PG, IJ]),
                        op=Alu.mult,
                    )
                    pos = outT[pi][:, ob + 16 : ob + 32]
                    nc.vector.tensor_reduce(
                        pos, P2.rearrange("p (g j) -> p g j", j=IJ), axis=AxX, op=Alu.add
                    )

                emit_preds_chunk(1, 1, [0, 1])
                emit_preds_chunk(1, 0, [0, 1])
                for bb in range(0, 8):
                    emit_dots(0, bb)
                emit_preds_chunk(1, 1, [2, 3])
                emit_preds_chunk(1, 0, [2, 3])
                for bb in range(8, 16):
                    emit_dots(0, bb)
                emit_preds_chunk(1, 1, [4])
                emit_preds_chunk(1, 0, [4, 5])
                emit_post(0, 0)
                for bb in range(0, 16):
                    emit_dots(1, bb)
                emit_post(0, 1)
                nc.sync.dma_start(out=out_d[0][:, :], in_=outT[0])
                emit_post(1, 0)
                emit_post(1, 1)
                nc.sync.dma_start(out=out_d[1][:, :], in_=outT[1])
            psPP.release()
            wkp.release()
            scr.release()

    nc.finalize()
    return nc


def _prep_inputs(encodings, hidden, W_ih, W_hh, b_ih, b_hh, Wk_w, Wk_b,
                 neg_rows, neg_cols):
    """Host-side reformat of the full inputs into per-core DMA-clean arrays."""
    f16 = np.float16
    enc = np.ascontiguousarray(encodings, dtype=np.float32)
    e6 = enc.reshape(NCORE, BS, C, C, PC_N, 128)  # (core, b, i, c, pc, pp)
    # GRU layout (r-major): [core, pp, r, pc, b*7+c]
    encT = np.ascontiguousarray(
        e6[:, :, :R].transpose(0, 5, 2, 4, 1, 3)   # (core, pp, r, pc, b, c)
    ).reshape(NCORE, 128, R * PC_N * BC).astype(f16)
    # dots layout: [core, pp, pc, b*49 + i*7 + c]
    encB = np.ascontiguousarray(
        e6.transpose(0, 5, 4, 1, 2, 3)   # (core, pp, pc, b, i, c)
    ).reshape(NCORE, 128, PC_N * BS * IJ).astype(f16)

    # m-major: [pp, m, pc, 128] so per-gate slices stream independently
    wih = np.ascontiguousarray(
        W_ih.reshape(6, 128, PC_N, 128).transpose(3, 0, 2, 1), dtype=np.float32
    ).reshape(128, 6 * P).astype(f16)
    whh = np.ascontiguousarray(
        W_hh.T.reshape(HC_N, 128, 768).transpose(1, 0, 2), dtype=np.float32
    ).reshape(128, HC_N * 768).astype(f16)
    wkh = np.ascontiguousarray(
        Wk_w.transpose(0, 2, 1).reshape(K, HC_N, 128, P).transpose(0, 2, 1, 3),
        dtype=np.float32,
    ).reshape(K, 128, HC_N * P).astype(f16)

    # H' = h + 1 bias folds
    rs = W_hh.astype(np.float32).sum(axis=1)              # [768]
    bsum = (b_ih + b_hh).astype(np.float32) - rs
    brz = bsum[:512].reshape(4, 128).T
    nbrz = (-bsum[256:512]).reshape(2, 128).T
    bhn = (b_hh.astype(np.float32) - rs)[512:].reshape(2, 128).T
    bin_ = b_ih[512:].astype(np.float32).reshape(2, 128).T

    bias_k = Wk_b.astype(np.float32) - Wk_w.astype(np.float32).sum(axis=2)  # [K, P]
    wkbT = np.ascontiguousarray(
        bias_k.reshape(K, PC_N, 128).transpose(2, 0, 1)
    ).reshape(128, K * PC_N)
    smA = np.ascontiguousarray(
        np.concatenate([brz, nbrz, bhn, bin_, -1.0 - wkbT, 1.0 - wkbT], axis=1),
        dtype=np.float32,
    )
    corr_k = np.einsum("kp,bijp->kbij", bias_k, enc, optimize=True).reshape(K, B, IJ)

    # negatives -> multiplicity counts over the 49 cells, plus the positive
    neg_idx = (neg_rows.astype(np.int64) * 7 + neg_cols.astype(np.int64))
    cnts = np.zeros((B, K, R, C, IJ), dtype=np.float32)
    np.add.at(
        cnts.reshape(B * K * R * C, IJ),
        (
            np.repeat(np.arange(B * K * R * C, dtype=np.int64), S - 1),
            neg_idx.reshape(-1),
        ),
        1.0,
    )
    cvec = np.arange(C)
    for k in range(K):
        for r in range(R):
            cnts[:, k, r, cvec, r * 7 + cvec] += 1.0   # the positive

    corr_dev, cnt_dev, posm_dev = [], [], []
    for pi, lst in enumerate(PASS_PAIRS):
        rows = PR[pi]
        cd = np.empty((NCORE, rows, BS, IJ), dtype=np.float32)
        nd = np.empty((NCORE, rows, BS, IJ), dtype=np.float32)
        pm = np.zeros((rows, IJ), dtype=np.float32)
        for qi, (k, r) in enumerate(lst):
            for c in range(C):
                row = qi * C + c
                cd[:, row] = corr_k[k].reshape(NCORE, BS, IJ)
                nd[:, row] = cnts[:, k, r, c].reshape(NCORE, BS, IJ)
                pm[row, r * 7 + c] = 1.0
        cd = cd + np.float32(MASK) * (nd == 0)
        corr_dev.append(cd.reshape(NCORE, rows, BS * IJ).astype(f16))
        cnt_dev.append(nd.reshape(NCORE, rows, BS * IJ).astype(f16))
        posm_dev.append(pm.astype(f16))

    posmAB = np.zeros((PR[1], 2 * IJ), dtype=f16)
    posmAB[: PR[0], :IJ] = posm_dev[0]
    posmAB[:, IJ:] = posm_dev[1]

    in_maps = []
    for core in range(NCORE):
        in_maps.append(
            {
                "encT": encT[core],
                "encB": encB[core],
                "wih": wih,
                "whh": whh,
                "wk": wkh,
                "smA": smA,
                "corr0": corr_dev[0][core],
                "corr1": corr_dev[1][core],
                "cnt0": cnt_dev[0][core],
                "cnt1": cnt_dev[1][core],
                "posmAB": posmAB,
            }
        )
    return in_maps


def _get_program():
    if "nc" not in _CACHE:
        _CACHE["nc"] = _build_program()
    return _CACHE["nc"]


def run_on_device(in_maps, trace=False, tmpdir=None):
    from concourse.bass_utils import run_bass_kernel_spmd

    nc = _get_program()
    return run_bass_kernel_spmd(
        nc, in_maps, list(range(NCORE)), trace=trace, tmpdir=tmpdir
    )


def kernel(**inputs):
    in_maps = _prep_inputs(**inputs)
    res = run_on_device(in_maps)
    loss_sum = 0.0
    corr_sum = 0.0
    for core in range(NCORE):
        for pi in range(2):
            o = np.asarray(res.results[core][f"out{pi}"], dtype=np.float64)
            o = o.reshape(PR[pi], 2, IJ)
            se = o[:, :, 0:16]
            pos = o[:, :, 16:32]
            mxg = o[:, :, 32:48]
            mxp = o[:, :, 48:49]
            loss_sum += np.sum(np.log(se) + mxp - pos)
            corr_sum += np.sum(pos >= mxg)
    loss = np.float32(loss_sum / N_PREDS)
    acc = np.float32(corr_sum / N_PREDS)
    return loss, acc


# revision 49
# speedup vs baseline: 1.1923x; 1.0244x over previous
"""Trainium2 Bass kernel for nn_CDC_62646392980082 (GRU-CPC loss_fn), v2.

Contract: kernel(**inputs) takes the FULL unsharded inputs (numpy) and
returns the FULL output (loss, acc) exactly like the jax reference.

Strategy (8 NeuronCores, data-parallel over batch B=256 -> 32/core):
  - GRU gates fused in PSUM: gi (x@W_ih) and gh (h@W_hh) accumulate into
    the same PSUM bank per step; sigmoid reads PSUM directly on the
    scalar engine (no gi copies / adds on the DVE).
  - H' = h+1 reparameterization: h' = (n+1)(1-z) + z*h' with n+1 =
    2*sigmoid(2x); rank-1 corrections folded into biases host-side.
    Avoids tanh table swaps and one DVE op per gate step.
  - preds split into two passes by r: pass0 = {k2:r0-3, k3:r0-2,
    k4:r0-1} (63 rows), pass1 = {k0:r0-5, k1:r0-4} (77 rows), so
    pass0's dots can overlap the preds tail.
  - clip alternates engines per p-chunk: even chunks clip directly from
    PSUM on the DVE; odd chunks evacuate via scalar-engine Identity and
    clip fp16->fp16 on the DVE fast path.
  - negatives folded host-side into multiplicity counts; the cnt==0
    mask (-60000) is folded into the corr tensor; softmax shift uses a
    per-partition-row max so exp's bias port applies it for free.
"""

import sys

if "/opt/trn_rl_repo" not in sys.path:
    sys.path.insert(0, "/opt/trn_rl_repo")

import numpy as np

B, K, R, C, P, H, S = 256, 5, 6, 7, 1280, 256, 64
NCORE = 8
BS = B // NCORE            # 32 images per core
BC = BS * C                # 224 (b, c) columns
PC_N = P // 128            # 10 p-chunks
HC_N = H // 128            # 2 h-chunks
IJ = 49                    # 7x7 cells

# pass structure: pairs (k, r) grouped so pass0 finishes by GRU step 3
PASS_PAIRS = [
    [(2, 0), (2, 1), (2, 2), (2, 3), (3, 0), (3, 1), (3, 2), (4, 0), (4, 1)],
    [(0, 0), (0, 1), (0, 2), (0, 3), (0, 4), (0, 5),
     (1, 0), (1, 1), (1, 2), (1, 3), (1, 4)],
]
PR = [len(PASS_PAIRS[0]) * C, len(PASS_PAIRS[1]) * C]   # 63, 77 rows
ROW_OFF = {}
for _pi, _lst in enumerate(PASS_PAIRS):
    for _qi, _kr in enumerate(_lst):
        ROW_OFF[_kr] = (_pi, _qi * C)

# preds chunks: (pass, k, [r...]) with adjacent r, emitted after step max(r)
CHUNKS = [
    (0, 4, [0, 1]), (0, 3, [0, 1]),            # ready after step 1
    (0, 3, [2]), (1, 1, [0, 1]),               # after step 2
    (0, 2, [0, 1]), (0, 2, [2, 3]),
    (1, 0, [0, 1]), (1, 0, [2, 3]), (1, 1, [2, 3]),   # after step 3
    (1, 1, [4]),                               # after step 4
    (1, 0, [4, 5]),                            # after step 5
]
N_PREDS = 20 * B * C       # 35840 global predictions
MASK = np.float32(-60000.0)

_CACHE = {}


def _build_program():
    import concourse.bacc as bacc
    import concourse.mybir as mybir
    from concourse.tile import TileContext

    f32 = mybir.dt.float32
    f16 = mybir.dt.float16
    Alu = mybir.AluOpType
    Act = mybir.ActivationFunctionType
    AxX = mybir.AxisListType.X

    nc = bacc.Bacc()
    dp = nc.declare_dram_parameter
    encT = dp("encT", [128, R * PC_N * BC], f16, isOutput=False)   # r-major
    encB = dp("encB", [128, PC_N * BS * IJ], f16, isOutput=False)
    wih = dp("wih", [128, PC_N * 768], f16, isOutput=False)
    whh = dp("whh", [128, HC_N * 768], f16, isOutput=False)
    wk = dp("wk", [K, 128, HC_N * P], f16, isOutput=False)
    # [brz 4][nbrz 2][bhn 2][bin 2][wklo 50][wkhi 50]
    smA = dp("smA", [128, 110], f32, isOutput=False)
    corr0 = dp("corr0", [PR[0], BS * IJ], f16, isOutput=False)  # corr - mask
    corr1 = dp("corr1", [PR[1], BS * IJ], f16, isOutput=False)
    cnt0 = dp("cnt0", [PR[0], BS * IJ], f16, isOutput=False)
    cnt1 = dp("cnt1", [PR[1], BS * IJ], f16, isOutput=False)
    posmAB = dp("posmAB", [PR[1], 2 * IJ], f16, isOutput=False)
    # per (pass, half): [se 16][pos 16][mxg 16][mxp 1] in 49-col blocks
    out0 = dp("out0", [PR[0], 2 * IJ], f32, isOutput=True)
    out1 = dp("out1", [PR[1], 2 * IJ], f32, isOutput=True)
    corr_d = [corr0, corr1]
    cnt_d = [cnt0, cnt1]
    out_d = [out0, out1]

    with TileContext(nc, pool_alloc_mode="queue") as tc:
        with tc.tile_pool(name="pers", bufs=1) as pers:
            # ---- persistent small loads (combined to minimize dispatches) ----
            smA_t = pers.tile([128, 110], f32, name="smA")
            posm_b = pers.tile([PR[1], 2 * IJ], f16, name="posmAB")
            brz_t = smA_t[:, 0:4]
            nbrz_t = smA_t[:, 4:6]
            bhn_t = smA_t[:, 6:8]
            bin_t = smA_t[:, 8:10]
            wklo_t = smA_t[:, 10:60]
            wkhi_t = smA_t[:, 60:110]
            posm_t = [posm_b[: PR[0], 0:IJ], posm_b[:, IJ : 2 * IJ]]

            # dots-phase encodings: loaded early on the gpsimd/scalar queues
            encB_b = pers.tile([128, PC_N * BS * IJ], f16, name="encB_b")
            hw_ = PC_N * BS * IJ // 2
            nc.gpsimd.dma_start(out=encB_b[:, hw_:], in_=encB[:, hw_:])

            # GRU context (H' = h+1), per h-chunk: [128, r*224]
            ctx = [pers.tile([128, R * BC], f16, name=f"ctx{t}") for t in range(2)]
            h0 = pers.tile([128, BC], f16)
            nc.vector.memset(h0, 1.0)

            predsT = [
                [
                    pers.tile([128, BS * PR[pi]], f16, name=f"pt{pi}_{m}")
                    for m in range(PC_N)
                ]
                for pi in range(2)
            ]
            corr_t = [
                pers.tile([PR[pi], BS * IJ], f16, name=f"corr{pi}") for pi in range(2)
            ]

            # scratch pools
            scr = tc.alloc_tile_pool(name="scr", bufs=1)
            wkp = tc.alloc_tile_pool(name="wkp", bufs=1)
            psPP = tc.alloc_tile_pool(name="psPP", bufs=3, space="PSUM")

            wk_t = {}

            def load_wk(k):
                t = wkp.tile([128, HC_N * P], f16, tag="wk", bufs=3, name=f"wk{k}")
                nc.sync.dma_start(out=t, in_=wk[k, :, :])
                wk_t[k] = t

            # ---------- preds emission ----------
            def emit_preds_chunk(pi, k, rs):
                nq = len(rs)
                n = nq * BC
                for m in range(PC_N):
                    ps = psPP.tile([128, 512], f32, tag="pp", name=f"pp{pi}_{k}_{rs[0]}_{m}")
                    for hc in range(HC_N):
                        nc.tensor.matmul(
                            ps[:, :n],
                            wk_t[k][:, hc * P + m * 128 : hc * P + (m + 1) * 128],
                            ctx[hc][:, rs[0] * BC : (rs[0] + nq) * BC],
                            start=(hc == 0),
                            stop=(hc == HC_N - 1),
                        )
                    off = ROW_OFF[(k, rs[0])][1]
                    dst = (
                        predsT[pi][m]
                        .rearrange("p (b x) -> p b x", b=BS)[:, :, off : off + nq * C]
                        .rearrange("p b (q c) -> p q b c", q=nq)
                    )
                    lo = wklo_t[:, k * PC_N + m : k * PC_N + m + 1]
                    hi = wkhi_t[:, k * PC_N + m : k * PC_N + m + 1]
                    psv = ps[:, :n].rearrange("p (q b c) -> p q b c", q=nq, b=BS)
                    if m in (0, 2, 5, 7):
                        nc.vector.tensor_scalar(dst, psv, lo, hi, Alu.max, Alu.min)
                    else:
                        ev = scr.tile([128, 448], f16, tag="ev", bufs=3, name=f"ev{pi}{k}{rs[0]}{m}")
                        evs = ev[:, :n]
                        nc.scalar.activation(evs, ps[:, :n], Act.Identity)
                        nc.vector.tensor_scalar(
                            dst,
                            evs.rearrange("p (q b c) -> p q b c", q=nq, b=BS),
                            lo, hi, Alu.max, Alu.min,
                        )

            # ---- phase 1: GRU (fused gates) ----
            with (
                tc.tile_pool(name="p1", bufs=1) as p1,
                tc.tile_pool(name="psG", bufs=2, space="PSUM") as psG,
                tc.tile_pool(name="psH", bufs=2, space="PSUM") as psH,
            ):
                wih_b = p1.tile([128, PC_N * 768], f16, name="wih_b")
                whh_b = p1.tile([128, HC_N * 768], f16, name="whh_b")
                enc_b = p1.tile([128, R * PC_N * BC], f16, name="enc_b")
                # wih is m-major [m, pc, 128]; split across both HWDGE queues
                # ordered by first use: gin (m4/m5), then t0 (m0, m2), t1 (m1, m3)
                nc.scalar.dma_start(
                    out=enc_b[:, : PC_N * BC], in_=encT[:, : PC_N * BC]
                )
                nc.sync.dma_start(out=wih_b[:, 4 * P : 5 * P], in_=wih[:, 4 * P : 5 * P])
                nc.sync.dma_start(out=smA_t, in_=smA[:, :])
                nc.scalar.dma_start(out=wih_b[:, 5 * P : 6 * P], in_=wih[:, 5 * P : 6 * P])
                for m in (0, 2):
                    nc.sync.dma_start(
                        out=wih_b[:, m * P : (m + 1) * P], in_=wih[:, m * P : (m + 1) * P]
                    )
                nc.scalar.dma_start(
                    out=enc_b[:, PC_N * BC : 2 * PC_N * BC],
                    in_=encT[:, PC_N * BC : 2 * PC_N * BC],
                )
                nc.sync.dma_start(out=whh_b, in_=whh[:, :])
                nc.sync.dma_start(out=posm_b, in_=posmAB[:, :])
                for m in (1, 3):
                    nc.scalar.dma_start(
                        out=wih_b[:, m * P : (m + 1) * P], in_=wih[:, m * P : (m + 1) * P]
                    )
                for r in range(2, R):
                    sl = slice(r * PC_N * BC, (r + 1) * PC_N * BC)
                    nc.scalar.dma_start(out=enc_b[:, sl], in_=encT[:, sl])
                nc.scalar.dma_start(out=encB_b[:, :hw_], in_=encB[:, :hw_])
                encv = enc_b.rearrange("p (r pc x) -> p r pc x", r=R, pc=PC_N)

                def wih_s(pc, m):
                    return wih_b[:, m * P + pc * 128 : m * P + (pc + 1) * 128]

                def whh_s(hc, m):
                    return whh_b[:, hc * 768 + m * 128 : hc * 768 + (m + 1) * 128]

                gin = [p1.tile([128, R * BC], f16, name=f"gin{t}") for t in range(2)]

                def emit_gin_chunk(t, r0_, nr):
                    # gi for the n gate, steps r0_..r0_+nr-1
                    ps = psPP.tile([128, 512], f32, tag="pp", name=f"gin{t}_{r0_}")
                    for pc in range(PC_N):
                        nc.tensor.matmul(
                            ps[:, : nr * BC],
                            wih_s(pc, 4 + t),
                            encv[:, r0_ : r0_ + nr, pc : pc + 1, :],
                            start=(pc == 0),
                            stop=(pc == PC_N - 1),
                        )
                    nc.vector.tensor_scalar(
                        gin[t][:, r0_ * BC : (r0_ + nr) * BC],
                        ps[:, : nr * BC],
                        bin_t[:, t : t + 1], 0.0, Alu.add, Alu.add,
                    )

                load_wk(4)
                load_wk(3)
                load_wk(2)
                load_wk(1)
                load_wk(0)
                for pi in range(2):
                    nc.sync.dma_start(out=corr_t[pi], in_=corr_d[pi][:, :])

                def emit_gru_step(r):
                    hprev = [h0, h0] if r == 0 else [
                        ctx[t][:, (r - 1) * BC : r * BC] for t in range(2)
                    ]
                    gps = []
                    hps = []
                    for t in range(2):
                        ps = psG.tile([128, 448], f32, tag="g", name=f"g{r}_{t}")
                        for half, m in ((0, t), (1, 2 + t)):   # r gate, z gate
                            sl = ps[:, half * BC : (half + 1) * BC]
                            for pc in range(PC_N):
                                nc.tensor.matmul(
                                    sl, wih_s(pc, m),
                                    enc_b[:, (r * PC_N + pc) * BC : (r * PC_N + pc + 1) * BC],
                                    start=(pc == 0), stop=False,
                                )
                            for hc in range(HC_N):
                                nc.tensor.matmul(
                                    sl, whh_s(hc, m), hprev[hc],
                                    start=False, stop=(hc == HC_N - 1),
                                )
                        gps.append(ps)
                        ph = psH.tile([128, BC], f32, tag="h", name=f"h{r}_{t}")
                        for hc in range(HC_N):
                            nc.tensor.matmul(
                                ph, whh_s(hc, 4 + t), hprev[hc],
                                start=(hc == 0), stop=(hc == HC_N - 1),
                            )
                        hps.append(ph)
                    for t in range(2):
                        gr = gps[t][:, 0:BC]
                        gz = gps[t][:, BC : 2 * BC]
                        rt = scr.tile([128, BC], f16, tag="rt", bufs=2, name=f"rt{r}{t}")
                        nc.scalar.activation(rt, gr, Act.Sigmoid, bias=brz_t[:, t : t + 1])
                        zt = scr.tile([128, BC], f16, tag="zt", bufs=2, name=f"zt{r}{t}")
                        nc.scalar.activation(zt, gz, Act.Sigmoid, bias=brz_t[:, 2 + t : 3 + t])
                        z1 = scr.tile([128, BC], f16, tag="z1", bufs=2, name=f"z1{r}{t}")
                        nc.scalar.activation(
                            z1, gz, Act.Sigmoid, bias=nbrz_t[:, t : t + 1], scale=-1.0
                        )
                        tV = scr.tile([128, BC], f16, tag="tV", bufs=2, name=f"tV{r}{t}")
                        nc.vector.scalar_tensor_tensor(
                            tV, hps[t], bhn_t[:, t : t + 1], rt, op0=Alu.add, op1=Alu.mult
                        )
                        tW = scr.tile([128, BC], f16, tag="tW", bufs=2, name=f"tW{r}{t}")
                        nc.vector.tensor_tensor(
                            tW, tV, gin[t][:, r * BC : (r + 1) * BC], op=Alu.add
                        )
                        sv = scr.tile([128, BC], f16, tag="sv", bufs=2, name=f"sv{r}{t}")
                        nc.scalar.activation(sv, tW, Act.Sigmoid, scale=2.0)
                        a_ = scr.tile([128, BC], f16, tag="a_", bufs=2, name=f"a{r}{t}")
                        nc.vector.tensor_tensor(a_, sv, z1, op=Alu.mult)
                        b2 = scr.tile([128, BC], f16, tag="b2", bufs=2, name=f"b{r}{t}")
                        nc.vector.tensor_tensor(b2, zt, hprev[t], op=Alu.mult)
                        nc.vector.scalar_tensor_tensor(
                            ctx[t][:, r * BC : (r + 1) * BC],
                            a_, 2.0, b2, op0=Alu.mult, op1=Alu.add,
                        )

                emit_gin_chunk(0, 0, 1)
                emit_gin_chunk(1, 0, 1)
                emit_gru_step(0)
                emit_gin_chunk(0, 1, 1)
                emit_gin_chunk(1, 1, 1)
                emit_gru_step(1)
                emit_gin_chunk(0, 2, 2)
                emit_gin_chunk(1, 2, 2)
                emit_preds_chunk(0, 4, [0, 1])
                emit_gru_step(2)
                emit_gin_chunk(0, 4, 2)
                emit_gin_chunk(1, 4, 2)
                emit_preds_chunk(0, 3, [0, 1])
                emit_gru_step(3)
                emit_preds_chunk(0, 3, [2])
                emit_preds_chunk(0, 2, [0, 1])
                emit_gru_step(4)
                emit_preds_chunk(0, 2, [2, 3])
                emit_gru_step(5)

            # ---- phase 3: pass-1 preds + dots + loss ----
            with (
                tc.tile_pool(name="p3", bufs=1) as p3,
                tc.tile_pool(name="psDP", bufs=4, space="PSUM") as psDP,
            ):
                cnt_t = [
                    p3.tile([PR[pi], BS * IJ], f16, name=f"cnt{pi}") for pi in range(2)
                ]
                D_t = [
                    p3.tile([PR[pi], BS * IJ], f16, name=f"D{pi}") for pi in range(2)
                ]
                outT = [
                    p3.tile([PR[pi], 2 * IJ], f32, name=f"outT{pi}") for pi in range(2)
                ]
                for pi in range(2):
                    nc.sync.dma_start(out=cnt_t[pi], in_=cnt_d[pi][:, :])

                def emit_dots(pi, bb):
                    rows = PR[pi]
                    ps = psDP.tile([rows, 2 * IJ], f32, tag="dp", name=f"dp{pi}_{bb}")
                    for half in range(2):
                        b = 2 * bb + half
                        for pc in range(PC_N):
                            nc.tensor.matmul(
                                ps[:, half * IJ : (half + 1) * IJ],
                                predsT[pi][pc][:, b * rows : (b + 1) * rows],
                                encB_b[:, pc * BS * IJ + b * IJ : pc * BS * IJ + (b + 1) * IJ],
                                start=(pc == 0),
                                stop=(pc == PC_N - 1),
                            )
                    csl = slice(2 * bb * IJ, (2 * bb + 2) * IJ)
                    nc.vector.tensor_tensor(
                        D_t[pi][:, csl], ps, corr_t[pi][:, csl], op=Alu.add
                    )

                PG = BS // 2   # 16 groups per post part

                def emit_post(pi, h):
                    rows = PR[pi]
                    c0 = h * PG * IJ
                    ob = h * IJ
                    Dp = D_t[pi][:, c0 : c0 + PG * IJ]
                    Dv = Dp.rearrange("p (g j) -> p g j", j=IJ)
                    mxg = outT[pi][:, ob + 32 : ob + 48]
                    nc.vector.tensor_reduce(mxg, Dv, axis=AxX, op=Alu.max)
                    mxp = outT[pi][:, ob + 48 : ob + 49]
                    nc.vector.tensor_reduce(mxp, mxg, axis=AxX, op=Alu.max)
                    nmx = scr.tile([rows, 1], f32, tag=f"nmx{pi}", bufs=2, name=f"nmx{pi}{h}")
                    nc.vector.tensor_scalar(nmx, mxp, -1.0, 0.0, Alu.mult, Alu.add)
                    B2 = p3.tile([rows, PG * IJ], f32, tag=f"B2{pi}", bufs=2, name=f"B2{pi}{h}")
                    nc.scalar.activation(B2, Dp, Act.Exp, bias=nmx[:, 0:1])
                    nc.vector.tensor_tensor(
                        B2, B2, cnt_t[pi][:, c0 : c0 + PG * IJ], op=Alu.mult
                    )
                    se = outT[pi][:, ob : ob + 16]
                    nc.vector.tensor_reduce(
                        se, B2.rearrange("p (g j) -> p g j", j=IJ), axis=AxX, op=Alu.add
                    )
                    P2 = p3.tile([rows, PG * IJ], f16, tag=f"P2{pi}", bufs=2, name=f"P2{pi}{h}")
                    nc.vector.tensor_tensor(
                        P2.rearrange("p (g j) -> p g j", j=IJ),
                        Dv,
                        posm_t[pi].unsqueeze(1).broadcast_to([rows, PG, IJ]),
                        op=Alu.mult,
                    )
                    pos = outT[pi][:, ob + 16 : ob + 32]
                    nc.vector.tensor_reduce(
                        pos, P2.rearrange("p (g j) -> p g j", j=IJ), axis=AxX, op=Alu.add
                    )

                emit_preds_chunk(1, 1, [0, 1])
                emit_preds_chunk(1, 0, [0, 1])
                for bb in range(0, 8):
                    emit_dots(0, bb)
                emit_preds_chunk(1, 1, [2, 3])
                emit_preds_chunk(1, 0, [2, 3])
                emit_post(0, 0)
                for bb in range(8, 16):
                    emit_dots(0, bb)
                emit_preds_chunk(1, 1, [4])
                emit_preds_chunk(1, 0, [4, 5])
                emit_post(0, 1)
                nc.sync.dma_start(out=out_d[0][:, :], in_=outT[0])
                for bb in range(0, 8):
                    emit_dots(1, bb)
                emit_post(1, 0)
                for bb in range(8, 16):
                    emit_dots(1, bb)
                emit_post(1, 1)
                nc.sync.dma_start(out=out_d[1][:, :], in_=outT[1])
            psPP.release()
            wkp.release()
            scr.release()

    nc.finalize()
    return nc


def _prep_inputs(encodings, hidden, W_ih, W_hh, b_ih, b_hh, Wk_w, Wk_b,
                 neg_rows, neg_cols):
    """Host-side reformat of the full inputs into per-core DMA-clean arrays."""
    f16 = np.float16
    enc = np.ascontiguousarray(encodings, dtype=np.float32)
    e6 = enc.reshape(NCORE, BS, C, C, PC_N, 128)  # (core, b, i, c, pc, pp)
    # GRU layout (r-major): [core, pp, r, pc, b*7+c]
    encT = np.ascontiguousarray(
        e6[:, :, :R].transpose(0, 5, 2, 4, 1, 3)   # (core, pp, r, pc, b, c)
    ).reshape(NCORE, 128, R * PC_N * BC).astype(f16)
    # dots layout: [core, pp, pc, b*49 + i*7 + c]
    encB = np.ascontiguousarray(
        e6.transpose(0, 5, 4, 1, 2, 3)   # (core, pp, pc, b, i, c)
    ).reshape(NCORE, 128, PC_N * BS * IJ).astype(f16)

    # m-major: [pp, m, pc, 128] so per-gate slices stream independently
    wih = np.ascontiguousarray(
        W_ih.reshape(6, 128, PC_N, 128).transpose(3, 0, 2, 1), dtype=np.float32
    ).reshape(128, 6 * P).astype(f16)
    whh = np.ascontiguousarray(
        W_hh.T.reshape(HC_N, 128, 768).transpose(1, 0, 2), dtype=np.float32
    ).reshape(128, HC_N * 768).astype(f16)
    wkh = np.ascontiguousarray(
        Wk_w.transpose(0, 2, 1).reshape(K, HC_N, 128, P).transpose(0, 2, 1, 3),
        dtype=np.float32,
    ).reshape(K, 128, HC_N * P).astype(f16)

    # H' = h + 1 bias folds
    rs = W_hh.astype(np.float32).sum(axis=1)              # [768]
    bsum = (b_ih + b_hh).astype(np.float32) - rs
    brz = bsum[:512].reshape(4, 128).T
    nbrz = (-bsum[256:512]).reshape(2, 128).T
    bhn = (b_hh.astype(np.float32) - rs)[512:].reshape(2, 128).T
    bin_ = b_ih[512:].astype(np.float32).reshape(2, 128).T

    bias_k = Wk_b.astype(np.float32) - Wk_w.astype(np.float32).sum(axis=2)  # [K, P]
    wkbT = np.ascontiguousarray(
        bias_k.reshape(K, PC_N, 128).transpose(2, 0, 1)
    ).reshape(128, K * PC_N)
    smA = np.ascontiguousarray(
        np.concatenate([brz, nbrz, bhn, bin_, -1.0 - wkbT, 1.0 - wkbT], axis=1),
        dtype=np.float32,
    )
    corr_k = np.einsum("kp,bijp->kbij", bias_k, enc, optimize=True).reshape(K, B, IJ)

    # negatives -> multiplicity counts over the 49 cells, plus the positive
    neg_idx = (neg_rows.astype(np.int64) * 7 + neg_cols.astype(np.int64))
    cnts = np.zeros((B, K, R, C, IJ), dtype=np.float32)
    np.add.at(
        cnts.reshape(B * K * R * C, IJ),
        (
            np.repeat(np.arange(B * K * R * C, dtype=np.int64), S - 1),
            neg_idx.reshape(-1),
        ),
        1.0,
    )
    cvec = np.arange(C)
    for k in range(K):
        for r in range(R):
            cnts[:, k, r, cvec, r * 7 + cvec] += 1.0   # the positive

    corr_dev, cnt_dev, posm_dev = [], [], []
    for pi, lst in enumerate(PASS_PAIRS):
        rows = PR[pi]
        cd = np.empty((NCORE, rows, BS, IJ), dtype=np.float32)
        nd = np.empty((NCORE, rows, BS, IJ), dtype=np.float32)
        pm = np.zeros((rows, IJ), dtype=np.float32)
        for qi, (k, r) in enumerate(lst):
            for c in range(C):
                row = qi * C + c
                cd[:, row] = corr_k[k].reshape(NCORE, BS, IJ)
                nd[:, row] = cnts[:, k, r, c].reshape(NCORE, BS, IJ)
                pm[row, r * 7 + c] = 1.0
        cd = cd + np.float32(MASK) * (nd == 0)
        corr_dev.append(cd.reshape(NCORE, rows, BS * IJ).astype(f16))
        cnt_dev.append(nd.reshape(NCORE, rows, BS * IJ).astype(f16))
        posm_dev.append(pm.astype(f16))

    posmAB = np.zeros((PR[1], 2 * IJ), dtype=f16)
    posmAB[: PR[0], :IJ] = posm_dev[0]
    posmAB[:, IJ:] = posm_dev[1]

    in_maps = []
    for core in range(NCORE):
        in_maps.append(
            {
                "encT": encT[core],
                "encB": encB[core],
                "wih": wih,
                "whh": whh,
                "wk": wkh,
                "smA": smA,
                "corr0": corr_dev[0][core],
                "corr1": corr_dev[1][core],
                "cnt0": cnt_dev[0][core],
                "cnt1": cnt_dev[1][core],
                "posmAB": posmAB,
            }
        )
    return in_maps


def _get_program():
    if "nc" not in _CACHE:
        _CACHE["nc"] = _build_program()
    return _CACHE["nc"]


def run_on_device(in_maps, trace=False, tmpdir=None):
    from concourse.bass_utils import run_bass_kernel_spmd

    nc = _get_program()
    return run_bass_kernel_spmd(
        nc, in_maps, list(range(NCORE)), trace=trace, tmpdir=tmpdir
    )


def kernel(**inputs):
    in_maps = _prep_inputs(**inputs)
    res = run_on_device(in_maps)
    loss_sum = 0.0
    corr_sum = 0.0
    for core in range(NCORE):
        for pi in range(2):
            o = np.asarray(res.results[core][f"out{pi}"], dtype=np.float64)
            o = o.reshape(PR[pi], 2, IJ)
            se = o[:, :, 0:16]
            pos = o[:, :, 16:32]
            mxg = o[:, :, 32:48]
            mxp = o[:, :, 48:49]
            loss_sum += np.sum(np.log(se) + mxp - pos)
            corr_sum += np.sum(pos >= mxg)
    loss = np.float32(loss_sum / N_PREDS)
    acc = np.float32(corr_sum / N_PREDS)
    return loss, acc
